# revision 19
# baseline (speedup 1.0000x reference)
"""Trainium2 Bass kernel for nn_ODEBlock (batch of 1024 independent neural-ODE
solves of a 4-dim state: 3 position dims + log-density).

Vector field per sample: for each of M=3 nuclei, z_i = x - c_i, r_i = |z_i|,
a_i = MLP(t, r_i) (2->64->64->64->1, tanh), F(x) = sum_i a_i z_i, and
dlogp/dt = -tr J = -sum_i (a'_i r_i + 3 a_i), with a' = dMLP/dr obtained by
exact forward-mode differentiation.

Integrator: classic RK4 with NSTEPS fixed steps (the reference's adaptive
dopri5 at tol 1e-3 is itself ~9e-6 from the true solution).

Distribution: pure data parallelism, 8 cores x 128 samples. Per core the
sample-world work (geometry, Newton sqrt, combine, RK updates) runs once on
all 128 partitions; only the MLP runs as NG independent groups so PE/ACT/DVE
overlap. Squares and 1-h^2 run on the otherwise idle GPSIMD engine.
"""
import numpy as np
from contextlib import ExitStack

import concourse.bacc as bacc
import concourse.bass as bass
import concourse.mybir as mybir
import concourse.tile as tile
from concourse import bass_utils

F32 = mybir.dt.float32
F32R = mybir.dt.float32r
U32 = mybir.dt.uint32
AF = mybir.ActivationFunctionType
ALU = mybir.AluOpType
AX = mybir.AxisListType

NCORES = 8
B = 1024
BC = B // NCORES          # samples per core
NG = 1                    # independent MLP groups per core
S = BC // NG              # samples per group
FD = 3 * S                # MLP free dim (3 nuclei x S samples)
NH = 64
NSTEPS = 2
HSTEP = 1.0 / NSTEPS
STAGE_C = (0.0, 0.5, 0.5, 1.0)
ACC_W = (1.0 / 6, 1.0 / 3, 1.0 / 3, 1.0 / 6)
NT = 2 * NSTEPS + 1
SQRT_MAGIC = 0x1FBD1DF5
NEWTON_ITERS = 1

_CACHE = {}


def _t_col(step, s):
    return 2 * step + (0 if s == 0 else (1 if s < 3 else 2))


def _emit_eval(nc, pools, consts, cur, K, tcol, tag):
    """One vector-field evaluation for all 128 samples."""
    wpool, mpool, ppool = pools
    (BD1, BD2, W3S, W0A, W0B, B0T, B1T, B2T, CREP, B3R, IDN, I2) = consts

    # ---- geometry (sample world, all samples at once) ----
    Z = wpool.tile([BC, 9], F32, tag="Z", name=f"Z{tag}")
    xb = cur[:, 0:3].unsqueeze(1).broadcast_to((BC, 3, 3))
    nc.vector.tensor_tensor(Z[:].rearrange("p (i k) -> p i k", k=3), xb,
                            CREP[:].rearrange("p (i k) -> p i k", k=3),
                            op=ALU.subtract)
    SQ = wpool.tile([BC, 9], F32, tag="SQ", name=f"SQ{tag}")
    nc.vector.tensor_tensor(SQ[:], Z[:], Z[:], op=ALU.mult)
    RSQ = wpool.tile([BC, 3], F32, tag="RSQ", name=f"RSQ{tag}")
    nc.vector.reduce_sum(RSQ[:], SQ[:].rearrange("p (i k) -> p i k", k=3),
                         axis=AX.X)
    # r = sqrt(rsq): bit-trick seed + Newton iterations (y <- (y + s/y)/2).
    # The final result lands strided at columns {0,32,64} of the persistent
    # [128, 65] R65 tile so ONE PE transpose yields rows at the partition
    # bases {0,32,64} that the layer-0 matmuls need.
    Yn = wpool.tile([BC, 3], F32, tag="Yn", name=f"Yn{tag}")
    nc.vector.tensor_scalar(Yn[:].bitcast(U32), RSQ[:].bitcast(U32),
                            1, None, op0=ALU.logical_shift_right)
    nc.vector.tensor_scalar(Yn[:].bitcast(U32), Yn[:].bitcast(U32),
                            SQRT_MAGIC, None, op0=ALU.add)
    Rc = wpool.tile([BC, 3], F32, tag="Rc", name=f"Rc{tag}")
    Dn = wpool.tile([BC, 3], F32, tag="Dn", name=f"Dn{tag}")
    Tn = wpool.tile([BC, 3], F32, tag="Tn", name=f"Tn{tag}")
    R3 = wpool.tile([BC, 3], F32, tag="R3", name=f"R3{tag}")
    for it in range(NEWTON_ITERS):
        nc.vector.reciprocal(Rc[:], Yn[:])
        nc.vector.tensor_tensor(Dn[:], RSQ[:], Rc[:], op=ALU.mult)
        nc.vector.tensor_tensor(Tn[:], Yn[:], Dn[:], op=ALU.add)
        out_t = R3 if it == NEWTON_ITERS - 1 else Yn
        nc.vector.tensor_scalar(out_t[:], Tn[:], 0.5, None, op0=ALU.mult)

    # ---- to MLP world: one [128,65] -> [65,128] PE transpose ----
    HD = [None] * NG
    for g in range(NG):
        gs = slice(S * g, S * (g + 1))
        PRT = ppool.tile([3, S], F32, tag=f"PR{g}", name=f"PRT{g}{tag}")
        nc.tensor.matmul(PRT[:], R3[gs, :], IDN[gs, gs],
                         is_transpose=True, start=True, stop=True)
        RTS = wpool.tile([3, S], F32, tag=f"RTS{g}", name=f"RTS{g}{tag}")
        nc.scalar.activation(RTS[:], PRT[:], AF.Copy)
        # layer 0: W0A [3,128] covers nuclei 0,1; W0B [3,64] nucleus 2
        P0A = ppool.tile([128, S], F32, tag=f"PB{g}", name=f"P0A{g}{tag}")
        nc.tensor.matmul(P0A[:], W0A[:], RTS[:], start=True, stop=True)
        P0B = ppool.tile([NH, S], F32, tag=f"PA{g}", name=f"P0B{g}{tag}")
        nc.tensor.matmul(P0B[:], W0B[:], RTS[:], start=True, stop=True)
        bias0 = B0T[:, tcol:tcol + 1]
        HD0 = mpool.tile([128, FD], F32R, tag=f"HD0{g}", name=f"HD0_{g}{tag}")
        nc.scalar.activation(HD0[0:NH, 0:S], P0A[0:NH, :], AF.Tanh, bias=bias0)
        nc.scalar.activation(HD0[0:NH, S:2 * S], P0A[NH:128, :], AF.Tanh, bias=bias0)
        nc.scalar.activation(HD0[0:NH, 2 * S:3 * S], P0B[:], AF.Tanh, bias=bias0)
        SQT = mpool.tile([128, FD], F32, tag=f"SQT{g}", name=f"SQT0_{g}{tag}")
        nc.vector.tensor_tensor(SQT[0:NH, :], HD0[0:NH, :].bitcast(F32), HD0[0:NH, :].bitcast(F32),
                                op=ALU.mult)
        nc.vector.tensor_scalar(HD0[NH:128, :], SQT[0:NH, :], -1.0, 1.0,
                                op0=ALU.mult, op1=ALU.add)

        # ---- layers 1,2: one K=128 block-diagonal matmul each ----
        P1 = ppool.tile([128, FD], F32, tag=f"PB{g}", name=f"P1_{g}{tag}")
        nc.tensor.matmul(P1[:], BD1[:], HD0[:], start=True, stop=True)
        HD1 = mpool.tile([128, FD], F32R, tag=f"HD1{g}", name=f"HD1_{g}{tag}")
        nc.scalar.activation(HD1[0:NH, :], P1[0:NH, :], AF.Tanh, bias=B1T[:])
        SQT = mpool.tile([128, FD], F32, tag=f"SQT{g}", name=f"SQT1_{g}{tag}")
        nc.vector.tensor_tensor(SQT[0:NH, :], HD1[0:NH, :].bitcast(F32), HD1[0:NH, :].bitcast(F32),
                                op=ALU.mult)
        TMP = mpool.tile([128, FD], F32, tag=f"TMP{g}", name=f"TMP1_{g}{tag}")
        nc.vector.tensor_scalar(TMP[NH:128, :], SQT[0:NH, :], -1.0, 1.0,
                                op0=ALU.mult, op1=ALU.add)
        nc.vector.tensor_tensor(HD1[NH:128, :], TMP[NH:128, :], P1[NH:128, :],
                                op=ALU.mult)

        P2 = ppool.tile([128, FD], F32, tag=f"PA{g}", name=f"P2_{g}{tag}")
        nc.tensor.matmul(P2[:], BD2[:], HD1[:], start=True, stop=True)
        HD2 = mpool.tile([128, FD], F32R, tag=f"HD2{g}", name=f"HD2_{g}{tag}")
        nc.scalar.activation(HD2[0:NH, :], P2[0:NH, :], AF.Tanh, bias=B2T[:])
        SQT = mpool.tile([128, FD], F32, tag=f"SQT{g}", name=f"SQT2_{g}{tag}")
        nc.vector.tensor_tensor(SQT[0:NH, :], HD2[0:NH, :].bitcast(F32), HD2[0:NH, :].bitcast(F32),
                                op=ALU.mult)
        TMP = mpool.tile([128, FD], F32, tag=f"TMP{g}", name=f"TMP2_{g}{tag}")
        nc.vector.tensor_scalar(TMP[NH:128, :], SQT[0:NH, :], -1.0, 1.0,
                                op0=ALU.mult, op1=ALU.add)
        nc.vector.tensor_tensor(HD2[NH:128, :], TMP[NH:128, :], P2[NH:128, :],
                                op=ALU.mult)
        HD[g] = HD2

    # ---- layer 3: 6 small matmuls laying AA out as (nucleus, group, sample)
    AA = ppool.tile([2, NG * FD], F32, tag="PAA", name=f"AA{tag}")
    for g in range(NG):
        for i in range(3):
            nc.tensor.matmul(AA[:, i * BC + g * S:i * BC + (g + 1) * S],
                             W3S[:], HD[g][:, i * S:(i + 1) * S],
                             start=True, stop=True)
    AAS = wpool.tile([2, NG * FD], F32, tag="AAS", name=f"AAS{tag}")
    nc.scalar.activation(AAS[:], AA[:], AF.Copy)

    # ---- back to sample world: 3 transposes, both groups per call ----
    AAT = ppool.tile([BC, 6], F32, tag="PR0", name=f"AAT{tag}")
    for i in range(3):
        nc.tensor.matmul(AAT[:, 2 * i:2 * i + 2], AAS[0:2, i * BC:(i + 1) * BC],
                         I2[:], is_transpose=True, start=True, stop=True)

    # ---- combine: F = sum_i a_i z_i ; -trJ = -(sum a' r + 3 sum a) ----
    aat3 = AAT[:].rearrange("p (i c) -> p i c", c=2)
    acol = aat3[:, :, 0:1]
    apcol = aat3[:, :, 1:2]
    nc.vector.tensor_scalar(acol, acol, B3R[:], None, op0=ALU.add)
    AZ = wpool.tile([BC, 9], F32, tag="AZ", name=f"AZ{tag}")
    nc.vector.tensor_tensor(AZ[:].rearrange("p (i k) -> p i k", k=3),
                            Z[:].rearrange("p (i k) -> p i k", k=3),
                            acol.broadcast_to((BC, 3, 3)), op=ALU.mult)
    nc.vector.reduce_sum(K[:, 0:3], AZ[:].rearrange("p (i k) -> p k i", k=3),
                         axis=AX.X)
    SCR = wpool.tile([BC, 3], F32, tag="SCR", name=f"SCR{tag}")
    TRA = wpool.tile([BC, 1], F32, tag="TRA", name=f"TRA{tag}")
    nc.vector.tensor_tensor(SCR[:].unsqueeze(2), apcol, R3[:].unsqueeze(2), op=ALU.mult)
    nc.vector.reduce_sum(TRA[:], SCR[:], axis=AX.X)
    SAc = wpool.tile([BC, 1], F32, tag="SA", name=f"SA{tag}")
    nc.vector.reduce_sum(SAc[:],
                         AAT[:].rearrange("p (i c) -> p c i", c=2)[:, 0:1, :],
                         axis=AX.X)
    nc.vector.tensor_scalar(K[:, 3:4], SAc[:], -3.0, TRA[:],
                            op0=ALU.mult, op1=ALU.subtract)


def _build():
    nc = bacc.Bacc("TRN2", target_bir_lowering=False, debug=False)

    def din(name, shape):
        return nc.dram_tensor(name, shape, F32, kind="ExternalInput")

    d_st = din("st", [BC, 4])
    d_bd1 = din("bd1", [128, 128])
    d_bd2 = din("bd2", [128, 128])
    d_w3s = din("w3s", [128, 2])
    d_w0a = din("w0a", [3, 128])
    d_w0b = din("w0b", [3, NH])
    d_b0t = din("b0t", [NH, NT])
    d_b1t = din("b1t", [NH, 1])
    d_b2t = din("b2t", [NH, 1])
    d_crep = din("crep", [BC, 9])
    d_b3r = din("b3r", [BC, 1])
    d_idn = din("idn", [128, 128])
    d_i2 = din("i2", [2, 2])
    d_out = nc.dram_tensor("out", [BC, 4], F32, kind="ExternalOutput")

    with tile.TileContext(nc) as tc, ExitStack() as ctx:
        cpool = ctx.enter_context(tc.tile_pool(name="const", bufs=1))
        spool = ctx.enter_context(tc.tile_pool(name="state", bufs=2))
        wpool = ctx.enter_context(tc.tile_pool(name="work", bufs=2))
        mpool = ctx.enter_context(tc.tile_pool(name="mlp", bufs=2))
        ppool = ctx.enter_context(tc.tile_pool(name="ps", bufs=1, space="PSUM"))

        def load(dram, shape, dt=F32):
            t = cpool.tile(shape, dt, name=f"c_{dram.name}", tag=f"c_{dram.name}")
            src_ap = dram[:] if dt is F32 else dram[:].bitcast(dt)
            nc.sync.dma_start(t[:], src_ap)
            return t

        BD1 = load(d_bd1, [128, 128], F32R)
        BD2 = load(d_bd2, [128, 128], F32R)
        W3S = load(d_w3s, [128, 2], F32R)
        W0A = load(d_w0a, [3, 128])
        W0B = load(d_w0b, [3, NH])
        B0T = load(d_b0t, [NH, NT])
        B1T = load(d_b1t, [NH, 1])
        B2T = load(d_b2t, [NH, 1])
        CREP = load(d_crep, [BC, 9])
        B3R = load(d_b3r, [BC, 1])
        IDN = load(d_idn, [128, 128])
        I2 = load(d_i2, [2, 2])
        # warm the ACT table set (tanh/square) while input DMAs run
        warm = cpool.tile([1, 1], F32, name="warm", tag="warm")
        nc.vector.memset(warm[:], 0.5)
        nc.scalar.activation(warm[:], warm[:], AF.Tanh)
        nc.scalar.activation(warm[:], warm[:], AF.Square)

        X = spool.tile([BC, 4], F32, tag="X", name="X0")
        nc.sync.dma_start(X[:], d_st[:])

        pools = (wpool, mpool, ppool)
        consts = (BD1, BD2, W3S, W0A, W0B, B0T, B1T, B2T, CREP, B3R, IDN, I2)
        for step in range(NSTEPS):
            Kacc = spool.tile([BC, 4], F32, tag="Kacc", name=f"Kacc{step}")
            Yprev = None
            for s in range(4):
                tcol = _t_col(step, s)
                cur = X if s == 0 else Yprev
                K = spool.tile([BC, 4], F32, tag="K", name=f"K_{step}_{s}")
                Ynew = (spool.tile([BC, 4], F32, tag="Y", name=f"Y_{step}_{s}")
                        if s < 3 else None)
                _emit_eval(nc, pools, consts, cur, K, tcol, f"_{step}_{s}")
                w = ACC_W[s] * HSTEP
                if s == 0:
                    nc.vector.tensor_scalar(Kacc[:], K[:], w, None, op0=ALU.mult)
                else:
                    nc.vector.scalar_tensor_tensor(Kacc[:], K[:], w, Kacc[:],
                                                   op0=ALU.mult, op1=ALU.add)
                if s < 3:
                    c = STAGE_C[s + 1] * HSTEP
                    nc.vector.scalar_tensor_tensor(Ynew[:], K[:], c, X[:],
                                                   op0=ALU.mult, op1=ALU.add)
                Yprev = Ynew
            Xn = spool.tile([BC, 4], F32, tag="X", name=f"X{step + 1}")
            nc.vector.tensor_tensor(Xn[:], X[:], Kacc[:], op=ALU.add)
            X = Xn
        nc.sync.dma_start(d_out[:], X[:])

    nc.compile()
    return nc


def _host_inputs(states, W0, b0, W1, b1, W2, b2, W3, b3, nuclei):
    f32 = lambda a: np.ascontiguousarray(a, dtype=np.float32)
    W0, b0, W1, b1, W2, b2, W3, b3, nuclei = map(
        f32, (W0, b0, W1, b1, W2, b2, W3, b3, nuclei))
    bd1 = np.zeros((128, 128), np.float32)
    bd1[0:NH, 0:NH] = W1
    bd1[NH:128, NH:128] = W0[1][:, None] * W1
    bd2 = np.zeros((128, 128), np.float32)
    bd2[0:NH, 0:NH] = W2
    bd2[NH:128, NH:128] = W2
    w3s = np.zeros((128, 2), np.float32)
    w3s[0:NH, 0] = W3[:, 0]
    w3s[NH:128, 1] = W3[:, 0]
    w0a = np.zeros((3, 128), np.float32)
    w0a[0, 0:NH] = W0[1]
    w0a[1, NH:128] = W0[1]
    w0b = np.zeros((3, NH), np.float32)
    w0b[2, :] = W0[1]
    ts = np.arange(NT, dtype=np.float32) * (HSTEP / 2)
    b0t = np.ascontiguousarray(ts[None, :] * W0[0][:, None] + b0[:, None],
                               np.float32)
    crep = np.tile(nuclei.reshape(-1), (BC, 1)).astype(np.float32)
    b3r = np.full((BC, 1), b3[0], np.float32)
    common = {
        "bd1": bd1, "bd2": bd2, "w3s": w3s, "w0a": w0a, "w0b": w0b, "b0t": b0t,
        "b1t": f32(b1[:, None]), "b2t": f32(b2[:, None]),
        "crep": crep, "b3r": b3r,
        "idn": np.eye(128, dtype=np.float32), "i2": np.eye(2, dtype=np.float32),
    }
    shards = np.ascontiguousarray(states, np.float32).reshape(NCORES, BC, 4)
    return [{**common, "st": shards[c]} for c in range(NCORES)]


def kernel(states, W0, b0, W1, b1, W2, b2, W3, b3, nuclei, _trace=False):
    if "nc" not in _CACHE:
        _CACHE["nc"] = _build()
    nc = _CACHE["nc"]
    in_maps = _host_inputs(states, W0, b0, W1, b1, W2, b2, W3, b3, nuclei)
    res = bass_utils.run_bass_kernel_spmd(nc, in_maps, list(range(NCORES)),
                                          trace=_trace)
    out = np.concatenate([res.results[c]["out"] for c in range(NCORES)], axis=0)
    if _trace:
        _CACHE["last_results"] = res
    return out


# revision 20
# speedup vs baseline: 1.0873x; 1.0873x over previous
"""Trainium2 Bass kernel for nn_ODEBlock (batch of 1024 independent neural-ODE
solves of a 4-dim state: 3 position dims + log-density).

Vector field per sample: for each of M=3 nuclei, z_i = x - c_i, r_i = |z_i|,
a_i = MLP(t, r_i) (2->64->64->64->1, tanh), F(x) = sum_i a_i z_i, and
dlogp/dt = -tr J = -sum_i (a'_i r_i + 3 a_i), with a' = dMLP/dr obtained by
exact forward-mode differentiation.

Integrator: classic RK4 with NSTEPS fixed steps (the reference's adaptive
dopri5 at tol 1e-3 is itself ~9e-6 from the true solution).

Distribution: pure data parallelism, 8 cores x 128 samples. Per core the
sample-world work (geometry, Newton sqrt, combine, RK updates) runs once on
all 128 partitions; only the MLP runs as NG independent groups so PE/ACT/DVE
overlap. Squares and 1-h^2 run on the otherwise idle GPSIMD engine.
"""
import numpy as np
from contextlib import ExitStack

import concourse.bacc as bacc
import concourse.bass as bass
import concourse.mybir as mybir
import concourse.tile as tile
from concourse import bass_utils

F32 = mybir.dt.float32
F32R = mybir.dt.float32r
U32 = mybir.dt.uint32
AF = mybir.ActivationFunctionType
ALU = mybir.AluOpType
AX = mybir.AxisListType

NCORES = 8
B = 1024
BC = B // NCORES          # samples per core
NG = 2                    # independent MLP groups per core
S = BC // NG              # samples per group
FD = 3 * S                # MLP free dim (3 nuclei x S samples)
NH = 64
NSTEPS = 2
HSTEP = 1.0 / NSTEPS
STAGE_C = (0.0, 0.5, 0.5, 1.0)
ACC_W = (1.0 / 6, 1.0 / 3, 1.0 / 3, 1.0 / 6)
NT = 2 * NSTEPS + 1
SQRT_MAGIC = 0x1FBD1DF5
NEWTON_ITERS = 1

_CACHE = {}


def _t_col(step, s):
    return 2 * step + (0 if s == 0 else (1 if s < 3 else 2))


def _emit_eval(nc, pools, consts, cur, K, tcol, tag):
    """One vector-field evaluation for all 128 samples."""
    wpool, mpool, ppool = pools
    (BD1, BD2, W3S, W0A, W0B, B0T, B1T, B2T, CREP, B3R, IDN, I2) = consts

    # ---- geometry (sample world, all samples at once) ----
    Z = wpool.tile([BC, 9], F32, tag="Z", name=f"Z{tag}")
    xb = cur[:, 0:3].unsqueeze(1).broadcast_to((BC, 3, 3))
    nc.vector.tensor_tensor(Z[:].rearrange("p (i k) -> p i k", k=3), xb,
                            CREP[:].rearrange("p (i k) -> p i k", k=3),
                            op=ALU.subtract)
    SQ = wpool.tile([BC, 9], F32, tag="SQ", name=f"SQ{tag}")
    nc.vector.tensor_tensor(SQ[:], Z[:], Z[:], op=ALU.mult)
    RSQ = wpool.tile([BC, 3], F32, tag="RSQ", name=f"RSQ{tag}")
    nc.vector.reduce_sum(RSQ[:], SQ[:].rearrange("p (i k) -> p i k", k=3),
                         axis=AX.X)
    # r = sqrt(rsq): bit-trick seed + Newton iterations (y <- (y + s/y)/2).
    # The final result lands strided at columns {0,32,64} of the persistent
    # [128, 65] R65 tile so ONE PE transpose yields rows at the partition
    # bases {0,32,64} that the layer-0 matmuls need.
    Yn = wpool.tile([BC, 3], F32, tag="Yn", name=f"Yn{tag}")
    nc.vector.tensor_scalar(Yn[:].bitcast(U32), RSQ[:].bitcast(U32),
                            1, None, op0=ALU.logical_shift_right)
    nc.vector.tensor_scalar(Yn[:].bitcast(U32), Yn[:].bitcast(U32),
                            SQRT_MAGIC, None, op0=ALU.add)
    Rc = wpool.tile([BC, 3], F32, tag="Rc", name=f"Rc{tag}")
    Dn = wpool.tile([BC, 3], F32, tag="Dn", name=f"Dn{tag}")
    Tn = wpool.tile([BC, 3], F32, tag="Tn", name=f"Tn{tag}")
    R3 = wpool.tile([BC, 3], F32, tag="R3", name=f"R3{tag}")
    for it in range(NEWTON_ITERS):
        nc.vector.reciprocal(Rc[:], Yn[:])
        nc.vector.tensor_tensor(Dn[:], RSQ[:], Rc[:], op=ALU.mult)
        nc.vector.tensor_tensor(Tn[:], Yn[:], Dn[:], op=ALU.add)
        out_t = R3 if it == NEWTON_ITERS - 1 else Yn
        nc.vector.tensor_scalar(out_t[:], Tn[:], 0.5, None, op0=ALU.mult)

    # ---- to MLP world: one [128,65] -> [65,128] PE transpose ----
    HD = [None] * NG
    for g in range(NG):
        gs = slice(S * g, S * (g + 1))
        PRT = ppool.tile([3, S], F32, tag=f"PR{g}", name=f"PRT{g}{tag}")
        nc.tensor.matmul(PRT[:], R3[gs, :], IDN[gs, gs],
                         is_transpose=True, start=True, stop=True)
        RTS = wpool.tile([3, S], F32, tag=f"RTS{g}", name=f"RTS{g}{tag}")
        nc.scalar.activation(RTS[:], PRT[:], AF.Copy)
        # layer 0: W0A [3,128] covers nuclei 0,1; W0B [3,64] nucleus 2
        P0A = ppool.tile([128, S], F32, tag=f"PB{g}", name=f"P0A{g}{tag}")
        nc.tensor.matmul(P0A[:], W0A[:], RTS[:], start=True, stop=True)
        P0B = ppool.tile([NH, S], F32, tag=f"PA{g}", name=f"P0B{g}{tag}")
        nc.tensor.matmul(P0B[:], W0B[:], RTS[:], start=True, stop=True)
        bias0 = B0T[:, tcol:tcol + 1]
        HD0 = mpool.tile([128, FD], F32R, tag=f"HD0{g}", name=f"HD0_{g}{tag}")
        nc.scalar.activation(HD0[0:NH, 0:S], P0A[0:NH, :], AF.Tanh, bias=bias0)
        nc.scalar.activation(HD0[0:NH, S:2 * S], P0A[NH:128, :], AF.Tanh, bias=bias0)
        nc.scalar.activation(HD0[0:NH, 2 * S:3 * S], P0B[:], AF.Tanh, bias=bias0)
        SQT = mpool.tile([128, FD], F32, tag=f"SQT{g}", name=f"SQT0_{g}{tag}")
        nc.vector.tensor_tensor(SQT[0:NH, :], HD0[0:NH, :].bitcast(F32), HD0[0:NH, :].bitcast(F32),
                                op=ALU.mult)
        nc.vector.tensor_scalar(HD0[NH:128, :], SQT[0:NH, :], -1.0, 1.0,
                                op0=ALU.mult, op1=ALU.add)

        # ---- layers 1,2: one K=128 block-diagonal matmul each ----
        P1 = ppool.tile([128, FD], F32, tag=f"PB{g}", name=f"P1_{g}{tag}")
        nc.tensor.matmul(P1[:], BD1[:], HD0[:], start=True, stop=True)
        HD1 = mpool.tile([128, FD], F32R, tag=f"HD1{g}", name=f"HD1_{g}{tag}")
        nc.scalar.activation(HD1[0:NH, :], P1[0:NH, :], AF.Tanh, bias=B1T[:])
        SQT = mpool.tile([128, FD], F32, tag=f"SQT{g}", name=f"SQT1_{g}{tag}")
        nc.vector.tensor_tensor(SQT[0:NH, :], HD1[0:NH, :].bitcast(F32), HD1[0:NH, :].bitcast(F32),
                                op=ALU.mult)
        TMP = mpool.tile([128, FD], F32, tag=f"TMP{g}", name=f"TMP1_{g}{tag}")
        nc.vector.tensor_scalar(TMP[NH:128, :], SQT[0:NH, :], -1.0, 1.0,
                                op0=ALU.mult, op1=ALU.add)
        nc.vector.tensor_tensor(HD1[NH:128, :], TMP[NH:128, :], P1[NH:128, :],
                                op=ALU.mult)

        P2 = ppool.tile([128, FD], F32, tag=f"PA{g}", name=f"P2_{g}{tag}")
        nc.tensor.matmul(P2[:], BD2[:], HD1[:], start=True, stop=True)
        HD2 = mpool.tile([128, FD], F32R, tag=f"HD2{g}", name=f"HD2_{g}{tag}")
        nc.scalar.activation(HD2[0:NH, :], P2[0:NH, :], AF.Tanh, bias=B2T[:])
        SQT = mpool.tile([128, FD], F32, tag=f"SQT{g}", name=f"SQT2_{g}{tag}")
        nc.vector.tensor_tensor(SQT[0:NH, :], HD2[0:NH, :].bitcast(F32), HD2[0:NH, :].bitcast(F32),
                                op=ALU.mult)
        TMP = mpool.tile([128, FD], F32, tag=f"TMP{g}", name=f"TMP2_{g}{tag}")
        nc.vector.tensor_scalar(TMP[NH:128, :], SQT[0:NH, :], -1.0, 1.0,
                                op0=ALU.mult, op1=ALU.add)
        nc.vector.tensor_tensor(HD2[NH:128, :], TMP[NH:128, :], P2[NH:128, :],
                                op=ALU.mult)
        HD[g] = HD2

    # ---- layer 3: 6 small matmuls laying AA out as (nucleus, group, sample)
    AA = ppool.tile([2, NG * FD], F32, tag="PAA", name=f"AA{tag}")
    for g in range(NG):
        for i in range(3):
            nc.tensor.matmul(AA[:, i * BC + g * S:i * BC + (g + 1) * S],
                             W3S[:], HD[g][:, i * S:(i + 1) * S],
                             start=True, stop=True)
    AAS = wpool.tile([2, NG * FD], F32, tag="AAS", name=f"AAS{tag}")
    nc.scalar.activation(AAS[:], AA[:], AF.Copy)

    # ---- back to sample world: 3 transposes, both groups per call ----
    AAT = ppool.tile([BC, 6], F32, tag="PR0", name=f"AAT{tag}")
    for i in range(3):
        nc.tensor.matmul(AAT[:, 2 * i:2 * i + 2], AAS[0:2, i * BC:(i + 1) * BC],
                         I2[:], is_transpose=True, start=True, stop=True)

    # ---- combine: F = sum_i a_i z_i ; -trJ = -(sum a' r + 3 sum a) ----
    aat3 = AAT[:].rearrange("p (i c) -> p i c", c=2)
    acol = aat3[:, :, 0:1]
    apcol = aat3[:, :, 1:2]
    nc.vector.tensor_scalar(acol, acol, B3R[:], None, op0=ALU.add)
    AZ = wpool.tile([BC, 9], F32, tag="AZ", name=f"AZ{tag}")
    nc.vector.tensor_tensor(AZ[:].rearrange("p (i k) -> p i k", k=3),
                            Z[:].rearrange("p (i k) -> p i k", k=3),
                            acol.broadcast_to((BC, 3, 3)), op=ALU.mult)
    nc.vector.reduce_sum(K[:, 0:3], AZ[:].rearrange("p (i k) -> p k i", k=3),
                         axis=AX.X)
    SCR = wpool.tile([BC, 3], F32, tag="SCR", name=f"SCR{tag}")
    TRA = wpool.tile([BC, 1], F32, tag="TRA", name=f"TRA{tag}")
    nc.vector.tensor_tensor(SCR[:].unsqueeze(2), apcol, R3[:].unsqueeze(2), op=ALU.mult)
    nc.vector.reduce_sum(TRA[:], SCR[:], axis=AX.X)
    SAc = wpool.tile([BC, 1], F32, tag="SA", name=f"SA{tag}")
    nc.vector.reduce_sum(SAc[:],
                         AAT[:].rearrange("p (i c) -> p c i", c=2)[:, 0:1, :],
                         axis=AX.X)
    nc.vector.tensor_scalar(K[:, 3:4], SAc[:], -3.0, TRA[:],
                            op0=ALU.mult, op1=ALU.subtract)


def _build():
    nc = bacc.Bacc("TRN2", target_bir_lowering=False, debug=False)

    def din(name, shape):
        return nc.dram_tensor(name, shape, F32, kind="ExternalInput")

    d_st = din("st", [BC, 4])
    d_bd1 = din("bd1", [128, 128])
    d_bd2 = din("bd2", [128, 128])
    d_w3s = din("w3s", [128, 2])
    d_w0a = din("w0a", [3, 128])
    d_w0b = din("w0b", [3, NH])
    d_b0t = din("b0t", [NH, NT])
    d_b1t = din("b1t", [NH, 1])
    d_b2t = din("b2t", [NH, 1])
    d_crep = din("crep", [BC, 9])
    d_b3r = din("b3r", [BC, 1])
    d_idn = din("idn", [128, 128])
    d_i2 = din("i2", [2, 2])
    d_out = nc.dram_tensor("out", [BC, 4], F32, kind="ExternalOutput")

    with tile.TileContext(nc) as tc, ExitStack() as ctx:
        cpool = ctx.enter_context(tc.tile_pool(name="const", bufs=1))
        spool = ctx.enter_context(tc.tile_pool(name="state", bufs=2))
        wpool = ctx.enter_context(tc.tile_pool(name="work", bufs=2))
        mpool = ctx.enter_context(tc.tile_pool(name="mlp", bufs=2))
        ppool = ctx.enter_context(tc.tile_pool(name="ps", bufs=1, space="PSUM"))

        def load(dram, shape, dt=F32):
            t = cpool.tile(shape, dt, name=f"c_{dram.name}", tag=f"c_{dram.name}")
            src_ap = dram[:] if dt is F32 else dram[:].bitcast(dt)
            nc.sync.dma_start(t[:], src_ap)
            return t

        BD1 = load(d_bd1, [128, 128], F32R)
        BD2 = load(d_bd2, [128, 128], F32R)
        W3S = load(d_w3s, [128, 2], F32R)
        W0A = load(d_w0a, [3, 128])
        W0B = load(d_w0b, [3, NH])
        B0T = load(d_b0t, [NH, NT])
        B1T = load(d_b1t, [NH, 1])
        B2T = load(d_b2t, [NH, 1])
        CREP = load(d_crep, [BC, 9])
        B3R = load(d_b3r, [BC, 1])
        IDN = load(d_idn, [128, 128])
        I2 = load(d_i2, [2, 2])
        # warm the ACT table set (tanh/square) while input DMAs run
        warm = cpool.tile([1, 1], F32, name="warm", tag="warm")
        nc.vector.memset(warm[:], 0.5)
        nc.scalar.activation(warm[:], warm[:], AF.Tanh)
        nc.scalar.activation(warm[:], warm[:], AF.Square)

        X = spool.tile([BC, 4], F32, tag="X", name="X0")
        nc.sync.dma_start(X[:], d_st[:])

        pools = (wpool, mpool, ppool)
        consts = (BD1, BD2, W3S, W0A, W0B, B0T, B1T, B2T, CREP, B3R, IDN, I2)
        for step in range(NSTEPS):
            Kacc = spool.tile([BC, 4], F32, tag="Kacc", name=f"Kacc{step}")
            Yprev = None
            for s in range(4):
                tcol = _t_col(step, s)
                cur = X if s == 0 else Yprev
                K = spool.tile([BC, 4], F32, tag="K", name=f"K_{step}_{s}")
                Ynew = (spool.tile([BC, 4], F32, tag="Y", name=f"Y_{step}_{s}")
                        if s < 3 else None)
                _emit_eval(nc, pools, consts, cur, K, tcol, f"_{step}_{s}")
                w = ACC_W[s] * HSTEP
                if s == 0:
                    nc.vector.tensor_scalar(Kacc[:], K[:], w, None, op0=ALU.mult)
                else:
                    nc.vector.scalar_tensor_tensor(Kacc[:], K[:], w, Kacc[:],
                                                   op0=ALU.mult, op1=ALU.add)
                if s < 3:
                    c = STAGE_C[s + 1] * HSTEP
                    nc.vector.scalar_tensor_tensor(Ynew[:], K[:], c, X[:],
                                                   op0=ALU.mult, op1=ALU.add)
                Yprev = Ynew
            Xn = spool.tile([BC, 4], F32, tag="X", name=f"X{step + 1}")
            nc.vector.tensor_tensor(Xn[:], X[:], Kacc[:], op=ALU.add)
            X = Xn
        nc.sync.dma_start(d_out[:], X[:])

    nc.compile()
    return nc


def _host_inputs(states, W0, b0, W1, b1, W2, b2, W3, b3, nuclei):
    f32 = lambda a: np.ascontiguousarray(a, dtype=np.float32)
    W0, b0, W1, b1, W2, b2, W3, b3, nuclei = map(
        f32, (W0, b0, W1, b1, W2, b2, W3, b3, nuclei))
    bd1 = np.zeros((128, 128), np.float32)
    bd1[0:NH, 0:NH] = W1
    bd1[NH:128, NH:128] = W0[1][:, None] * W1
    bd2 = np.zeros((128, 128), np.float32)
    bd2[0:NH, 0:NH] = W2
    bd2[NH:128, NH:128] = W2
    w3s = np.zeros((128, 2), np.float32)
    w3s[0:NH, 0] = W3[:, 0]
    w3s[NH:128, 1] = W3[:, 0]
    w0a = np.zeros((3, 128), np.float32)
    w0a[0, 0:NH] = W0[1]
    w0a[1, NH:128] = W0[1]
    w0b = np.zeros((3, NH), np.float32)
    w0b[2, :] = W0[1]
    ts = np.arange(NT, dtype=np.float32) * (HSTEP / 2)
    b0t = np.ascontiguousarray(ts[None, :] * W0[0][:, None] + b0[:, None],
                               np.float32)
    crep = np.tile(nuclei.reshape(-1), (BC, 1)).astype(np.float32)
    b3r = np.full((BC, 1), b3[0], np.float32)
    common = {
        "bd1": bd1, "bd2": bd2, "w3s": w3s, "w0a": w0a, "w0b": w0b, "b0t": b0t,
        "b1t": f32(b1[:, None]), "b2t": f32(b2[:, None]),
        "crep": crep, "b3r": b3r,
        "idn": np.eye(128, dtype=np.float32), "i2": np.eye(2, dtype=np.float32),
    }
    shards = np.ascontiguousarray(states, np.float32).reshape(NCORES, BC, 4)
    return [{**common, "st": shards[c]} for c in range(NCORES)]


def kernel(states, W0, b0, W1, b1, W2, b2, W3, b3, nuclei, _trace=False):
    if "nc" not in _CACHE:
        _CACHE["nc"] = _build()
    nc = _CACHE["nc"]
    in_maps = _host_inputs(states, W0, b0, W1, b1, W2, b2, W3, b3, nuclei)
    res = bass_utils.run_bass_kernel_spmd(nc, in_maps, list(range(NCORES)),
                                          trace=_trace)
    out = np.concatenate([res.results[c]["out"] for c in range(NCORES)], axis=0)
    if _trace:
        _CACHE["last_results"] = res
    return out
